# revision 23
# baseline (speedup 1.0000x reference)
"""GATv2 attention-score kernel for 8 Trainium2 NeuronCores.

Reference computation (per b, h):
    scores[i, j] = sum_d silu(q[i, d] + k[j, d]) * a[h, d]
    attn = softmax(where(mask, -inf, scores), axis=-1), zeroed at mask.

Algorithm: low-rank separable expansion of the silu kernel.  On
[-xm, xm]^2 the bivariate function silu(x + y) admits
    silu(x + y) = c(x) + sum_t g_t(x) * h_t(y) + eps
(c(x) is free: softmax over j is invariant to per-row additive terms,
so the SVD is taken on the row-centered kernel; R=8 leaves eps giving
~5e-3 end-to-end rel err vs the 2e-2 gate).  Then

    scores[i, j] ~ const_i + sum_{t,d} [g_t(q[i,d]) a_d] * [h_t(k[j,d])]

i.e. one (L x 64R) @ (64R x L) matmul per (b, h) — the whole O(L^2 D)
silu stage runs on the TensorEngine instead of ScalarE (the baseline's
109 us ScalarE-roofline silu is gone entirely).

Per-core dataflow (B=4, H=8, LQ=LK=256, D=64, R=8, 4 (b,h) pairs/core,
all sharing one b so the mask is per-core constant):
  - Host: evaluate factor tensors UT[(t,d), i] = g_t(q[i,d]) a_d and
    VT[(t,d), j] = h_t(k[j,d]) by linear interp on a 1025-point grid
    (lower-order host work: O(L D R) per pair vs O(L^2 D) on device).
    Factors t0..t3 ship fp16, t4..t7 fp8e4m3, all four pairs packed
    into ONE uint8 DRAM tensor (12 KB/partition) so a rep issues just
    1 input DMA + 1 output DMA: HWDGE descriptor-gen is ~500 ns per
    dma_start and serializes on the SP engine, which made DMA count
    (20/rep in v1) the dominant cost; fewer+bigger transfers measured
    strictly faster at every step (20 -> 5 -> 2 DMAs per rep).
  - TensorE: per pair, one full-bank (128, 512) PSUM tile; per i-tile
    of 128 queries a 4-matmul accumulation group (K=128, N=256,
    fp16/fp8 via bitcast views), then a single N=512 identity-weight
    matmul injects the premasked mask tile (-60000 at padded) into
    both halves at once.
  - ScalarE: one Exp activation per pair straight out of PSUM
    (bias +3; centered scores are in [-2.1, 2.1]), fp16 output.
  - DVE: per-segment reduce_sum -> reciprocal -> tensor_scalar
    (ex * recip * 1024) into a per-rep fp16 out tile; the x1024 keeps
    small attn values out of fp16-subnormal range (host divides it
    back out).  One output DMA per rep.
"""

import numpy as np

B, H, L, D = 4, 8, 256, 64
NCORES = 8
BH = 4            # (b, h) pairs per core
R = 8             # separable rank of the silu kernel
NCH = R // 2      # 128-partition contraction chunks (2 factors each)
NF16 = 2          # chunks 0..NF16-1 in fp16, rest fp8e4m3
GRID_N = 1025
EXP_BIAS = 3.0
OUT_SCALE = 1024.0
MASK_NEG = -60000.0

# uv byte layout per partition: [uth 2*L*2B | vth 2*L*2B | utq 2*L | vtq 2*L]
UTH_OFF = 0
VTH_OFF = 2 * L * 2
UTQ_OFF = VTH_OFF + 2 * L * 2
VTQ_OFF = UTQ_OFF + 2 * L
UV_BYTES = VTQ_OFF + 2 * L          # 3072

_cache = {}


def _factors(xm):
    """Row-centered SVD factors of silu(x+y) on [-xm, xm]^2."""
    key = ("fac", round(xm, 3))
    if key in _cache:
        return _cache[key]
    x = np.linspace(-xm, xm, GRID_N)
    s = x[:, None] + x[None, :]
    F = (s / (1.0 + np.exp(-s)))
    F -= F.mean(axis=1, keepdims=True)
    U, sv, Vt = np.linalg.svd(F)
    G = U[:, :R] * np.sqrt(sv[:R])
    Hf = (Vt[:R] * np.sqrt(sv[:R])[:, None]).T
    for t in range(R):
        al = np.sqrt(np.abs(Hf[:, t]).max() / np.abs(G[:, t]).max())
        G[:, t] *= al
        Hf[:, t] /= al
    _cache[key] = (x, G, Hf)
    return _cache[key]


def _interp_all(vals, x, table):
    """table lookup with linear interp: vals (N,) -> (N, R)."""
    hstep = x[1] - x[0]
    f = (vals - x[0]) / hstep
    i0 = np.clip(f.astype(np.int64), 0, len(x) - 2)
    frac = np.clip(f - i0, 0.0, 1.0)[:, None]
    return table[i0] * (1.0 - frac) + table[i0 + 1] * frac


def _build_program(reps=1, ndev=NCORES, stages="full", gps_reduce=False,
                   ident_first=False, lp_sums=False):
    """stages: "dma" (input DMAs + const out DMA), "mm" (+matmuls,
    exp but no DVE), "full" (everything).  For HW bisection."""
    import concourse.mybir as mybir
    from concourse import bacc
    from concourse.tile import TileContext

    F32 = mybir.dt.float32
    F16 = mybir.dt.float16
    F8 = mybir.dt.float8e4
    U8 = mybir.dt.uint8
    nc = bacc.Bacc("TRN2", target_bir_lowering=False, debug=False,
                   num_devices=ndev)

    uv_d = nc.dram_tensor("uv", [128, BH * UV_BYTES], U8,
                          kind="ExternalInput")
    mm_d = nc.dram_tensor("mm", [128, 2 * L], F16, kind="ExternalInput")
    id_d = nc.dram_tensor("ident", [128, 128], F16, kind="ExternalInput")
    out_d = nc.dram_tensor("out", [128, BH * 2 * L], F16,
                           kind="ExternalOutput")
    tok_d = nc.dram_tensor("tok", [128, 2], F32, kind="ExternalOutput")

    with TileContext(nc) as tc:
        with (
            tc.tile_pool(name="const", bufs=1) as c_pool,
            tc.tile_pool(name="io", bufs=3) as io_pool,
            tc.tile_pool(name="ex", bufs=3) as ex_pool,
            tc.tile_pool(name="sm", bufs=3) as sm_pool,
            tc.tile_pool(name="outp", bufs=2) as out_pool,
            tc.tile_pool(name="psum", bufs=4, space="PSUM") as ps_pool,
        ):
            mm_t = c_pool.tile([128, 2 * L], F16, tag="mm")
            nc.sync.dma_start(mm_t[:], mm_d[:])
            id_t = c_pool.tile([128, 128], F16, tag="id")
            nc.sync.dma_start(id_t[:], id_d[:])
            bias_t = c_pool.tile([128, 1], F32, tag="bias")
            nc.vector.memset(bias_t[:], EXP_BIAS)

            for _rep in range(reps):
                if stages == "full":
                    out_t = out_pool.tile([128, BH * 2 * L], F16, tag="out")
                uv_t = io_pool.tile([128, BH * UV_BYTES], U8, tag="uv")
                nc.sync.dma_start(uv_t[:], uv_d[:])
                if ident_first and stages == "full":
                    ps_list = []
                    for l in range(BH):
                        psl = ps_pool.tile([128, 2 * L], F32, tag="ps")
                        ps_list.append(psl)
                        nc.tensor.matmul(
                            psl[:], lhsT=id_t[:], rhs=mm_t[:],
                            start=True, stop=False, skip_group_check=True)
                for l in range(BH):
                    lo = l * UV_BYTES
                    if stages == "dma":
                        if l == BH - 1:
                            nc.sync.dma_start(
                                out_d[:].bitcast(U8)[:, :UV_BYTES],
                                uv_t[:, lo:lo + UV_BYTES])
                        continue
                    uth = uv_t[:, lo + UTH_OFF:lo + VTH_OFF].bitcast(F16)
                    vth = uv_t[:, lo + VTH_OFF:lo + UTQ_OFF].bitcast(F16)
                    utq = uv_t[:, lo + UTQ_OFF:lo + VTQ_OFF].bitcast(F8)
                    vtq = uv_t[:, lo + VTQ_OFF:lo + UV_BYTES].bitcast(F8)

                    ex_t = ex_pool.tile([128, 2 * L], F16, tag="ex")
                    sums = sm_pool.tile([128, 2], F16 if lp_sums else F32,
                                        tag="sums")
                    recip = sm_pool.tile([128, 2], F32, tag="recip")

                    if ident_first:
                        ps = ps_list[l]
                    else:
                        ps = ps_pool.tile([128, 2 * L], F32, tag="ps")
                        nc.tensor.matmul(
                            ps[:], lhsT=id_t[:], rhs=mm_t[:],
                            start=True, stop=False, skip_group_check=True)
                    for it in range(2):
                        for c in range(NCH):
                            if c < NF16:
                                ut_, vt_, cc = uth, vth, c
                            else:
                                ut_, vt_, cc = utq, vtq, c - NF16
                            nc.tensor.matmul(
                                ps[:, it * L:(it + 1) * L],
                                lhsT=ut_[:, cc * L + it * 128:
                                         cc * L + it * 128 + 128],
                                rhs=vt_[:, cc * L:(cc + 1) * L],
                                start=False, stop=(c == NCH - 1),
                                skip_group_check=True)
                    nc.scalar.activation(
                        ex_t[:], ps[:],
                        mybir.ActivationFunctionType.Exp,
                        bias=bias_t[:])
                    if stages == "mm":
                        if l == BH - 1:
                            nc.sync.dma_start(out_d[:, :2 * L], ex_t[:])
                        continue
                    red = nc.gpsimd if gps_reduce else nc.vector
                    if lp_sums:
                        with nc.allow_low_precision("fp16 softmax denom"):
                            red.reduce_sum(
                                sums[:],
                                ex_t[:].rearrange("p (s j) -> p s j", j=L),
                                axis=mybir.AxisListType.X)
                    else:
                        red.reduce_sum(
                            sums[:],
                            ex_t[:].rearrange("p (s j) -> p s j", j=L),
                            axis=mybir.AxisListType.X)
                    nc.vector.reciprocal(recip[:], sums[:])
                    for it in range(2):
                        nc.vector.tensor_scalar(
                            out_t[:, (l * 2 + it) * L:(l * 2 + it + 1) * L],
                            ex_t[:, it * L:(it + 1) * L],
                            recip[:, it:it + 1], OUT_SCALE,
                            op0=mybir.AluOpType.mult,
                            op1=mybir.AluOpType.mult)
                if stages == "full":
                    nc.sync.dma_start(out_d[:], out_t[:])
            # tiny completion token (one DMA after the last rep): lets the
            # bench wait on execution end by fetching ~1 KB instead of MBs
            if stages == "full":
                if lp_sums:
                    nc.sync.dma_start(tok_d[:].bitcast(F16)[:, :2], sums[:])
                else:
                    nc.sync.dma_start(tok_d[:], sums[:])
            elif stages == "mm":
                nc.sync.dma_start(tok_d[:].bitcast(F16), ex_t[:, :4])
            else:
                nc.sync.dma_start(tok_d[:].bitcast(U8), uv_t[:, :8])

    nc.compile()
    return nc


def _prep_core_inputs(q, k, mask, attention):
    """Host-side prep: packed factor tensors + premasked mask tile."""
    import concourse.mybir as mybir
    F8NP = mybir.dt.np(mybir.dt.float8e4)

    q = np.asarray(q, np.float32)
    k = np.asarray(k, np.float32)
    a = np.asarray(attention, np.float32).reshape(H, D)
    mask = np.asarray(mask).reshape(B, L, L)

    xm = float(max(np.abs(q).max(), np.abs(k).max())) + 0.05
    x, G, Hf = _factors(xm)

    gq = _interp_all(q.ravel(), x, G).reshape(B, H, L, D, R)
    hk = _interp_all(k.ravel(), x, Hf).reshape(B, H, L, D, R)

    in_maps = []
    for core in range(NCORES):
        uv = np.empty((128, BH * UV_BYTES), np.uint8)
        for l in range(BH):
            f = BH * core + l
            b, h = f // H, f % H
            # (R, D, L) -> rows (t*64+d), cols i
            UT = (gq[b, h] * a[h][None, :, None]).transpose(2, 1, 0)
            VT = hk[b, h].transpose(2, 1, 0)
            UT = UT.reshape(R * D, L)
            VT = VT.reshape(R * D, L)
            # fp16 chunks 0..NF16-1: (128, NF16*L) partition-major
            uth = np.concatenate(
                [UT[c * 128:(c + 1) * 128] for c in range(NF16)],
                axis=1).astype(np.float16)
            vth = np.concatenate(
                [VT[c * 128:(c + 1) * 128] for c in range(NF16)],
                axis=1).astype(np.float16)
            utq = np.concatenate(
                [UT[c * 128:(c + 1) * 128] for c in range(NF16, NCH)],
                axis=1).astype(F8NP)
            vtq = np.concatenate(
                [VT[c * 128:(c + 1) * 128] for c in range(NF16, NCH)],
                axis=1).astype(F8NP)
            uv[:, l * UV_BYTES + UTH_OFF:l * UV_BYTES + VTH_OFF] = uth.view(np.uint8)
            uv[:, l * UV_BYTES + VTH_OFF:l * UV_BYTES + UTQ_OFF] = vth.view(np.uint8)
            uv[:, l * UV_BYTES + UTQ_OFF:l * UV_BYTES + VTQ_OFF] = utq.view(np.uint8)
            uv[:, l * UV_BYTES + VTQ_OFF:l * UV_BYTES + UV_BYTES] = vtq.view(np.uint8)
        b0 = BH * core // H
        mb = np.where(mask[b0], np.float16(MASK_NEG), np.float16(0))
        mm = np.concatenate([mb[:128], mb[128:]], axis=1).astype(np.float16)
        in_maps.append({
            "uv": uv,
            "mm": np.ascontiguousarray(mm),
            "ident": np.eye(128, dtype=np.float16),
        })
    return in_maps


def _get_runner():
    """Persistent jitted shard_map runner over 8 cores."""
    if "runner" in _cache:
        return _cache["runner"]

    import jax
    import concourse.mybir as mybir
    from jax.sharding import Mesh, PartitionSpec
    from jax.experimental.shard_map import shard_map
    from concourse import bass2jax

    bass2jax.install_neuronx_cc_hook()
    nc = _build_program()

    part_name = (nc.partition_id_tensor.name
                 if nc.partition_id_tensor else None)
    in_names, out_names, out_avals, zero_outs = [], [], [], []
    for alloc in nc.m.functions[0].allocations:
        if not isinstance(alloc, mybir.MemoryLocationSet):
            continue
        name = alloc.memorylocations[0].name
        if alloc.kind == "ExternalInput":
            if name != part_name:
                in_names.append(name)
        elif alloc.kind == "ExternalOutput":
            shape = tuple(alloc.tensor_shape)
            dtype = mybir.dt.np(alloc.dtype)
            out_names.append(name)
            out_avals.append(jax.core.ShapedArray(shape, dtype))
            zero_outs.append(np.zeros(shape, dtype))
    n_params = len(in_names)
    all_names = in_names + out_names
    if part_name is not None:
        all_names = all_names + [part_name]

    def _body(*args):
        operands = list(args)
        if part_name is not None:
            operands.append(bass2jax.partition_id_tensor())
        return tuple(bass2jax._bass_exec_p.bind(
            *operands,
            out_avals=tuple(out_avals),
            in_names=tuple(all_names),
            out_names=tuple(out_names),
            lowering_input_output_aliases=(),
            sim_require_finite=True,
            sim_require_nnan=True,
            nc=nc,
        ))

    devices = jax.devices()[:NCORES]
    mesh = Mesh(np.asarray(devices), ("core",))
    n_outs = len(out_names)
    sharded = jax.jit(
        shard_map(_body, mesh=mesh,
                  in_specs=(PartitionSpec("core"),) * (n_params + n_outs),
                  out_specs=(PartitionSpec("core"),) * n_outs,
                  check_rep=False),
        donate_argnums=tuple(range(n_params, n_params + n_outs)),
        keep_unused=True)

    def run(in_maps):
        concat_in = [
            np.concatenate([in_maps[c][nm] for c in range(NCORES)], axis=0)
            for nm in in_names]
        concat_zeros = [np.zeros((NCORES * z.shape[0], *z.shape[1:]), z.dtype)
                        for z in zero_outs]
        outs = sharded(*concat_in, *concat_zeros)
        return [
            {nm: np.asarray(outs[i]).reshape(NCORES, *out_avals[i].shape)[c]
             for i, nm in enumerate(out_names)}
            for c in range(NCORES)]

    run.sharded = sharded
    run.in_names = in_names
    run.zero_outs = zero_outs
    _cache["runner"] = run
    return run


def kernel(q, k, scale, mask, attention):
    results = _get_runner()(_prep_core_inputs(q, k, mask, attention))
    attn = np.empty((B, H, L, L), np.float32)
    inv = np.float32(1.0 / OUT_SCALE)
    for core in range(NCORES):
        o = results[core]["out"]                  # (128, BH*2*L) f16
        of = o.astype(np.float32) * inv
        for l in range(BH):
            f = BH * core + l
            b, h = f // H, f % H
            attn[b, h, :128] = of[:, (l * 2) * L:(l * 2 + 1) * L]
            attn[b, h, 128:] = of[:, (l * 2 + 1) * L:(l * 2 + 2) * L]
    return attn


# revision 25
# speedup vs baseline: 1.1589x; 1.1589x over previous
"""GATv2 attention-score kernel for 8 Trainium2 NeuronCores.

Reference computation (per b, h):
    scores[i, j] = sum_d silu(q[i, d] + k[j, d]) * a[h, d]
    attn = softmax(where(mask, -inf, scores), axis=-1), zeroed at mask.

Algorithm: low-rank separable expansion of the silu kernel.  On
[-xm, xm]^2 the bivariate function silu(x + y) admits
    silu(x + y) = c(x) + sum_t g_t(x) * h_t(y) + eps
(c(x) is free: softmax over j is invariant to per-row additive terms,
so the SVD is taken on the row-centered kernel; R=8 leaves eps giving
~5e-3 end-to-end rel err vs the 2e-2 gate).  Then

    scores[i, j] ~ const_i + sum_{t,d} [g_t(q[i,d]) a_d] * [h_t(k[j,d])]

i.e. one (L x 64R) @ (64R x L) matmul per (b, h) — the whole O(L^2 D)
silu stage runs on the TensorEngine instead of ScalarE (the baseline's
109 us ScalarE-roofline silu is gone entirely).

Per-core dataflow (B=4, H=8, LQ=LK=256, D=64, R=8, 4 (b,h) pairs/core,
all sharing one b so the mask is per-core constant):
  - Host: evaluate factor tensors UT[(t,d), i] = g_t(q[i,d]) a_d and
    VT[(t,d), j] = h_t(k[j,d]) by linear interp on a 1025-point grid
    (lower-order host work: O(L D R) per pair vs O(L^2 D) on device).
    Factors t0..t3 ship fp16, t4..t7 fp8e4m3, all four pairs packed
    into ONE uint8 DRAM tensor (12 KB/partition) so a rep issues just
    1 input DMA + 1 output DMA: HWDGE descriptor-gen is ~500 ns per
    dma_start and serializes on the SP engine, which made DMA count
    (20/rep in v1) the dominant cost; fewer+bigger transfers measured
    strictly faster at every step (20 -> 5 -> 2 DMAs per rep).
  - TensorE: per pair, one full-bank (128, 512) PSUM tile.  A single
    N=512 identity-weight matmul injects the premasked mask tile
    (-60000 at padded) into both halves first (start=True), then per
    i-tile of 128 queries: two fp16 matmuls (K=128, N=256) plus ONE
    DoubleRow fp8 matmul fusing both fp8 chunks (K=128x2 k-tiles via
    3D APs) -- 25%% fewer PE stream cycles, bit-identical in CoreSim,
    measured ~1.4 us faster than the unfused form.
  - ScalarE: one Exp activation per pair straight out of PSUM
    (bias +3; centered scores are in [-2.1, 2.1]), fp16 output.
  - DVE: per-segment reduce_sum -> reciprocal -> tensor_scalar
    (ex * recip * 1024) into a per-rep fp16 out tile; the x1024 keeps
    small attn values out of fp16-subnormal range (host divides it
    back out).  One output DMA per rep.
"""

import numpy as np

B, H, L, D = 4, 8, 256, 64
NCORES = 8
BH = 4            # (b, h) pairs per core
R = 8             # separable rank of the silu kernel
NCH = R // 2      # 128-partition contraction chunks (2 factors each)
NF16 = 2          # chunks 0..NF16-1 in fp16, rest fp8e4m3
GRID_N = 1025
EXP_BIAS = 3.0
OUT_SCALE = 1024.0
MASK_NEG = -60000.0

# uv byte layout per partition: [uth 2*L*2B | vth 2*L*2B | utq 2*L | vtq 2*L]
UTH_OFF = 0
VTH_OFF = 2 * L * 2
UTQ_OFF = VTH_OFF + 2 * L * 2
VTQ_OFF = UTQ_OFF + 2 * L
UV_BYTES = VTQ_OFF + 2 * L          # 3072

_cache = {}


def _factors(xm):
    """Row-centered SVD factors of silu(x+y) on [-xm, xm]^2."""
    key = ("fac", round(xm, 3))
    if key in _cache:
        return _cache[key]
    x = np.linspace(-xm, xm, GRID_N)
    s = x[:, None] + x[None, :]
    F = (s / (1.0 + np.exp(-s)))
    F -= F.mean(axis=1, keepdims=True)
    U, sv, Vt = np.linalg.svd(F)
    G = U[:, :R] * np.sqrt(sv[:R])
    Hf = (Vt[:R] * np.sqrt(sv[:R])[:, None]).T
    for t in range(R):
        al = np.sqrt(np.abs(Hf[:, t]).max() / np.abs(G[:, t]).max())
        G[:, t] *= al
        Hf[:, t] /= al
    _cache[key] = (x, G, Hf)
    return _cache[key]


def _interp_all(vals, x, table):
    """table lookup with linear interp: vals (N,) -> (N, R)."""
    hstep = x[1] - x[0]
    f = (vals - x[0]) / hstep
    i0 = np.clip(f.astype(np.int64), 0, len(x) - 2)
    frac = np.clip(f - i0, 0.0, 1.0)[:, None]
    return table[i0] * (1.0 - frac) + table[i0 + 1] * frac


def _build_program(reps=1, ndev=NCORES, stages="full", gps_reduce=False,
                   ident_first=False, lp_sums=False, dr=True):
    """stages: "dma" (input DMAs + const out DMA), "mm" (+matmuls,
    exp but no DVE), "full" (everything).  For HW bisection."""
    import concourse.mybir as mybir
    from concourse import bacc
    from concourse.tile import TileContext

    F32 = mybir.dt.float32
    F16 = mybir.dt.float16
    F8 = mybir.dt.float8e4
    U8 = mybir.dt.uint8
    nc = bacc.Bacc("TRN2", target_bir_lowering=False, debug=False,
                   num_devices=ndev)

    uv_d = nc.dram_tensor("uv", [128, BH * UV_BYTES], U8,
                          kind="ExternalInput")
    mm_d = nc.dram_tensor("mm", [128, 2 * L], F16, kind="ExternalInput")
    id_d = nc.dram_tensor("ident", [128, 128], F16, kind="ExternalInput")
    out_d = nc.dram_tensor("out", [128, BH * 2 * L], F16,
                           kind="ExternalOutput")
    tok_d = nc.dram_tensor("tok", [128, 2], F32, kind="ExternalOutput")

    with TileContext(nc) as tc:
        with (
            tc.tile_pool(name="const", bufs=1) as c_pool,
            tc.tile_pool(name="io", bufs=3) as io_pool,
            tc.tile_pool(name="ex", bufs=3) as ex_pool,
            tc.tile_pool(name="sm", bufs=3) as sm_pool,
            tc.tile_pool(name="outp", bufs=2) as out_pool,
            tc.tile_pool(name="psum", bufs=4, space="PSUM") as ps_pool,
        ):
            mm_t = c_pool.tile([128, 2 * L], F16, tag="mm")
            nc.sync.dma_start(mm_t[:], mm_d[:])
            id_t = c_pool.tile([128, 128], F16, tag="id")
            nc.sync.dma_start(id_t[:], id_d[:])
            bias_t = c_pool.tile([128, 1], F32, tag="bias")
            nc.vector.memset(bias_t[:], EXP_BIAS)

            for _rep in range(reps):
                if stages == "full":
                    out_t = out_pool.tile([128, BH * 2 * L], F16, tag="out")
                uv_t = io_pool.tile([128, BH * UV_BYTES], U8, tag="uv")
                nc.sync.dma_start(uv_t[:], uv_d[:])
                if ident_first and stages == "full":
                    ps_list = []
                    for l in range(BH):
                        psl = ps_pool.tile([128, 2 * L], F32, tag="ps")
                        ps_list.append(psl)
                        nc.tensor.matmul(
                            psl[:], lhsT=id_t[:], rhs=mm_t[:],
                            start=True, stop=False, skip_group_check=True)
                for l in range(BH):
                    lo = l * UV_BYTES
                    if stages == "dma":
                        if l == BH - 1:
                            nc.sync.dma_start(
                                out_d[:].bitcast(U8)[:, :UV_BYTES],
                                uv_t[:, lo:lo + UV_BYTES])
                        continue
                    uth = uv_t[:, lo + UTH_OFF:lo + VTH_OFF].bitcast(F16)
                    vth = uv_t[:, lo + VTH_OFF:lo + UTQ_OFF].bitcast(F16)
                    utq = uv_t[:, lo + UTQ_OFF:lo + VTQ_OFF].bitcast(F8)
                    vtq = uv_t[:, lo + VTQ_OFF:lo + UV_BYTES].bitcast(F8)

                    ex_t = ex_pool.tile([128, 2 * L], F16, tag="ex")
                    sums = sm_pool.tile([128, 2], F16 if lp_sums else F32,
                                        tag="sums")
                    recip = sm_pool.tile([128, 2], F32, tag="recip")

                    if ident_first:
                        ps = ps_list[l]
                    else:
                        ps = ps_pool.tile([128, 2 * L], F32, tag="ps")
                        nc.tensor.matmul(
                            ps[:], lhsT=id_t[:], rhs=mm_t[:],
                            start=True, stop=False, skip_group_check=True)
                    for it in range(2):
                        for c in range(NF16):
                            nc.tensor.matmul(
                                ps[:, it * L:(it + 1) * L],
                                lhsT=uth[:, c * L + it * 128:
                                         c * L + it * 128 + 128],
                                rhs=vth[:, c * L:(c + 1) * L],
                                start=False,
                                stop=(not dr and False),
                                skip_group_check=True)
                        if dr:
                            # fp8 chunks 2+3 fused: K = 128 partitions x
                            # 2 k-tiles, one DoubleRow matmul
                            wv = utq.rearrange("p (kt i) -> p kt i", kt=2)
                            rv = vtq.rearrange("p (kt j) -> p kt j", kt=2)
                            nc.tensor.matmul(
                                ps[:, it * L:(it + 1) * L],
                                lhsT=wv[:, :, it * 128:it * 128 + 128],
                                rhs=rv[:, :, :],
                                start=False, stop=True,
                                perf_mode=mybir.MatmulPerfMode.DoubleRow,
                                skip_group_check=True)
                        else:
                            for c in range(NF16, NCH):
                                cc = c - NF16
                                nc.tensor.matmul(
                                    ps[:, it * L:(it + 1) * L],
                                    lhsT=utq[:, cc * L + it * 128:
                                             cc * L + it * 128 + 128],
                                    rhs=vtq[:, cc * L:(cc + 1) * L],
                                    start=False, stop=(c == NCH - 1),
                                    skip_group_check=True)
                    nc.scalar.activation(
                        ex_t[:], ps[:],
                        mybir.ActivationFunctionType.Exp,
                        bias=bias_t[:])
                    if stages == "mm":
                        if l == BH - 1:
                            nc.sync.dma_start(out_d[:, :2 * L], ex_t[:])
                        continue
                    red = nc.gpsimd if gps_reduce else nc.vector
                    if lp_sums:
                        with nc.allow_low_precision("fp16 softmax denom"):
                            red.reduce_sum(
                                sums[:],
                                ex_t[:].rearrange("p (s j) -> p s j", j=L),
                                axis=mybir.AxisListType.X)
                    else:
                        red.reduce_sum(
                            sums[:],
                            ex_t[:].rearrange("p (s j) -> p s j", j=L),
                            axis=mybir.AxisListType.X)
                    nc.vector.reciprocal(recip[:], sums[:])
                    for it in range(2):
                        nc.vector.tensor_scalar(
                            out_t[:, (l * 2 + it) * L:(l * 2 + it + 1) * L],
                            ex_t[:, it * L:(it + 1) * L],
                            recip[:, it:it + 1], OUT_SCALE,
                            op0=mybir.AluOpType.mult,
                            op1=mybir.AluOpType.mult)
                if stages == "full":
                    nc.sync.dma_start(out_d[:], out_t[:])
            # tiny completion token (one DMA after the last rep): lets the
            # bench wait on execution end by fetching ~1 KB instead of MBs
            if stages == "full":
                if lp_sums:
                    nc.sync.dma_start(tok_d[:].bitcast(F16)[:, :2], sums[:])
                else:
                    nc.sync.dma_start(tok_d[:], sums[:])
            elif stages == "mm":
                nc.sync.dma_start(tok_d[:].bitcast(F16), ex_t[:, :4])
            else:
                nc.sync.dma_start(tok_d[:].bitcast(U8), uv_t[:, :8])

    nc.compile()
    return nc


def _prep_core_inputs(q, k, mask, attention):
    """Host-side prep: packed factor tensors + premasked mask tile."""
    import concourse.mybir as mybir
    F8NP = mybir.dt.np(mybir.dt.float8e4)

    q = np.asarray(q, np.float32)
    k = np.asarray(k, np.float32)
    a = np.asarray(attention, np.float32).reshape(H, D)
    mask = np.asarray(mask).reshape(B, L, L)

    xm = float(max(np.abs(q).max(), np.abs(k).max())) + 0.05
    x, G, Hf = _factors(xm)

    gq = _interp_all(q.ravel(), x, G).reshape(B, H, L, D, R)
    hk = _interp_all(k.ravel(), x, Hf).reshape(B, H, L, D, R)

    in_maps = []
    for core in range(NCORES):
        uv = np.empty((128, BH * UV_BYTES), np.uint8)
        for l in range(BH):
            f = BH * core + l
            b, h = f // H, f % H
            # (R, D, L) -> rows (t*64+d), cols i
            UT = (gq[b, h] * a[h][None, :, None]).transpose(2, 1, 0)
            VT = hk[b, h].transpose(2, 1, 0)
            UT = UT.reshape(R * D, L)
            VT = VT.reshape(R * D, L)
            # fp16 chunks 0..NF16-1: (128, NF16*L) partition-major
            uth = np.concatenate(
                [UT[c * 128:(c + 1) * 128] for c in range(NF16)],
                axis=1).astype(np.float16)
            vth = np.concatenate(
                [VT[c * 128:(c + 1) * 128] for c in range(NF16)],
                axis=1).astype(np.float16)
            utq = np.concatenate(
                [UT[c * 128:(c + 1) * 128] for c in range(NF16, NCH)],
                axis=1).astype(F8NP)
            vtq = np.concatenate(
                [VT[c * 128:(c + 1) * 128] for c in range(NF16, NCH)],
                axis=1).astype(F8NP)
            uv[:, l * UV_BYTES + UTH_OFF:l * UV_BYTES + VTH_OFF] = uth.view(np.uint8)
            uv[:, l * UV_BYTES + VTH_OFF:l * UV_BYTES + UTQ_OFF] = vth.view(np.uint8)
            uv[:, l * UV_BYTES + UTQ_OFF:l * UV_BYTES + VTQ_OFF] = utq.view(np.uint8)
            uv[:, l * UV_BYTES + VTQ_OFF:l * UV_BYTES + UV_BYTES] = vtq.view(np.uint8)
        b0 = BH * core // H
        mb = np.where(mask[b0], np.float16(MASK_NEG), np.float16(0))
        mm = np.concatenate([mb[:128], mb[128:]], axis=1).astype(np.float16)
        in_maps.append({
            "uv": uv,
            "mm": np.ascontiguousarray(mm),
            "ident": np.eye(128, dtype=np.float16),
        })
    return in_maps


def _get_runner():
    """Persistent jitted shard_map runner over 8 cores."""
    if "runner" in _cache:
        return _cache["runner"]

    import jax
    import concourse.mybir as mybir
    from jax.sharding import Mesh, PartitionSpec
    from jax.experimental.shard_map import shard_map
    from concourse import bass2jax

    bass2jax.install_neuronx_cc_hook()
    nc = _build_program()

    part_name = (nc.partition_id_tensor.name
                 if nc.partition_id_tensor else None)
    in_names, out_names, out_avals, zero_outs = [], [], [], []
    for alloc in nc.m.functions[0].allocations:
        if not isinstance(alloc, mybir.MemoryLocationSet):
            continue
        name = alloc.memorylocations[0].name
        if alloc.kind == "ExternalInput":
            if name != part_name:
                in_names.append(name)
        elif alloc.kind == "ExternalOutput":
            shape = tuple(alloc.tensor_shape)
            dtype = mybir.dt.np(alloc.dtype)
            out_names.append(name)
            out_avals.append(jax.core.ShapedArray(shape, dtype))
            zero_outs.append(np.zeros(shape, dtype))
    n_params = len(in_names)
    all_names = in_names + out_names
    if part_name is not None:
        all_names = all_names + [part_name]

    def _body(*args):
        operands = list(args)
        if part_name is not None:
            operands.append(bass2jax.partition_id_tensor())
        return tuple(bass2jax._bass_exec_p.bind(
            *operands,
            out_avals=tuple(out_avals),
            in_names=tuple(all_names),
            out_names=tuple(out_names),
            lowering_input_output_aliases=(),
            sim_require_finite=True,
            sim_require_nnan=True,
            nc=nc,
        ))

    devices = jax.devices()[:NCORES]
    mesh = Mesh(np.asarray(devices), ("core",))
    n_outs = len(out_names)
    sharded = jax.jit(
        shard_map(_body, mesh=mesh,
                  in_specs=(PartitionSpec("core"),) * (n_params + n_outs),
                  out_specs=(PartitionSpec("core"),) * n_outs,
                  check_rep=False),
        donate_argnums=tuple(range(n_params, n_params + n_outs)),
        keep_unused=True)

    def run(in_maps):
        concat_in = [
            np.concatenate([in_maps[c][nm] for c in range(NCORES)], axis=0)
            for nm in in_names]
        concat_zeros = [np.zeros((NCORES * z.shape[0], *z.shape[1:]), z.dtype)
                        for z in zero_outs]
        outs = sharded(*concat_in, *concat_zeros)
        return [
            {nm: np.asarray(outs[i]).reshape(NCORES, *out_avals[i].shape)[c]
             for i, nm in enumerate(out_names)}
            for c in range(NCORES)]

    run.sharded = sharded
    run.in_names = in_names
    run.zero_outs = zero_outs
    _cache["runner"] = run
    return run


def kernel(q, k, scale, mask, attention):
    results = _get_runner()(_prep_core_inputs(q, k, mask, attention))
    attn = np.empty((B, H, L, L), np.float32)
    inv = np.float32(1.0 / OUT_SCALE)
    for core in range(NCORES):
        o = results[core]["out"]                  # (128, BH*2*L) f16
        of = o.astype(np.float32) * inv
        for l in range(BH):
            f = BH * core + l
            b, h = f // H, f % H
            attn[b, h, :128] = of[:, (l * 2) * L:(l * 2 + 1) * L]
            attn[b, h, 128:] = of[:, (l * 2 + 1) * L:(l * 2 + 2) * L]
    return attn


# revision 27
# speedup vs baseline: 1.1982x; 1.0339x over previous
"""GATv2 attention-score kernel for 8 Trainium2 NeuronCores.

Reference computation (per b, h):
    scores[i, j] = sum_d silu(q[i, d] + k[j, d]) * a[h, d]
    attn = softmax(where(mask, -inf, scores), axis=-1), zeroed at mask.

Algorithm: low-rank separable expansion of the silu kernel.  On
[-xm, xm]^2 the bivariate function silu(x + y) admits
    silu(x + y) = c(x) + sum_t g_t(x) * h_t(y) + eps
(c(x) is free: softmax over j is invariant to per-row additive terms,
so the SVD is taken on the row-centered kernel; R=8 leaves eps giving
~5e-3 end-to-end rel err vs the 2e-2 gate).  Then

    scores[i, j] ~ const_i + sum_{t,d} [g_t(q[i,d]) a_d] * [h_t(k[j,d])]

i.e. one (L x 64R) @ (64R x L) matmul per (b, h) — the whole O(L^2 D)
silu stage runs on the TensorEngine instead of ScalarE (the baseline's
109 us ScalarE-roofline silu is gone entirely).

Per-core dataflow (B=4, H=8, LQ=LK=256, D=64, R=8, 4 (b,h) pairs/core,
all sharing one b so the mask is per-core constant):
  - Host: evaluate factor tensors UT[(t,d), i] = g_t(q[i,d]) a_d and
    VT[(t,d), j] = h_t(k[j,d]) by linear interp on a 1025-point grid
    (lower-order host work: O(L D R) per pair vs O(L^2 D) on device).
    Factors t0..t3 ship fp16, t4..t7 fp8e4m3, all four pairs packed
    into ONE uint8 DRAM tensor (12 KB/partition) so a rep issues just
    1 input DMA + 1 output DMA: HWDGE descriptor-gen is ~500 ns per
    dma_start and serializes on the SP engine, which made DMA count
    (20/rep in v1) the dominant cost; fewer+bigger transfers measured
    strictly faster at every step (20 -> 5 -> 2 DMAs per rep).
  - TensorE: per pair, one full-bank (128, 512) PSUM tile.  A single
    N=512 identity-weight matmul injects the premasked mask tile
    (-60000 at padded) into both halves first (start=True), then per
    i-tile of 128 queries: two fp16 matmuls (K=128, N=256) plus ONE
    DoubleRow fp8 matmul fusing both fp8 chunks (K=128x2 k-tiles via
    3D APs) -- 25%% fewer PE stream cycles, bit-identical in CoreSim,
    measured ~1.4 us faster than the unfused form.
  - ScalarE: one Exp activation per pair straight out of PSUM
    (bias +3; centered scores are in [-2.1, 2.1]), fp16 output.
  - DVE: per-segment reduce_sum -> reciprocal -> tensor_scalar
    (ex * recip * 1024) into a per-rep fp16 out tile; the x1024 keeps
    small attn values out of fp16-subnormal range (host divides it
    back out).  One output DMA per rep.
  - Buffering: 6 PSUM banks + 4 input-DMA bufs; at 4 PSUM banks the
    next rep's first matmul group stalled on this rep's activations
    (measured 5.7us -> 3.4us from this change alone).
"""

import numpy as np

B, H, L, D = 4, 8, 256, 64
NCORES = 8
BH = 4            # (b, h) pairs per core
R = 8             # separable rank of the silu kernel
NCH = R // 2      # 128-partition contraction chunks (2 factors each)
NF16 = 2          # chunks 0..NF16-1 in fp16, rest fp8e4m3
GRID_N = 1025
EXP_BIAS = 3.0
OUT_SCALE = 1024.0
MASK_NEG = -60000.0

# uv byte layout per partition: [uth 2*L*2B | vth 2*L*2B | utq 2*L | vtq 2*L]
UTH_OFF = 0
VTH_OFF = 2 * L * 2
UTQ_OFF = VTH_OFF + 2 * L * 2
VTQ_OFF = UTQ_OFF + 2 * L
UV_BYTES = VTQ_OFF + 2 * L          # 3072

_cache = {}


def _factors(xm):
    """Row-centered SVD factors of silu(x+y) on [-xm, xm]^2."""
    key = ("fac", round(xm, 3))
    if key in _cache:
        return _cache[key]
    x = np.linspace(-xm, xm, GRID_N)
    s = x[:, None] + x[None, :]
    F = (s / (1.0 + np.exp(-s)))
    F -= F.mean(axis=1, keepdims=True)
    U, sv, Vt = np.linalg.svd(F)
    G = U[:, :R] * np.sqrt(sv[:R])
    Hf = (Vt[:R] * np.sqrt(sv[:R])[:, None]).T
    for t in range(R):
        al = np.sqrt(np.abs(Hf[:, t]).max() / np.abs(G[:, t]).max())
        G[:, t] *= al
        Hf[:, t] /= al
    _cache[key] = (x, G, Hf)
    return _cache[key]


def _interp_all(vals, x, table):
    """table lookup with linear interp: vals (N,) -> (N, R)."""
    hstep = x[1] - x[0]
    f = (vals - x[0]) / hstep
    i0 = np.clip(f.astype(np.int64), 0, len(x) - 2)
    frac = np.clip(f - i0, 0.0, 1.0)[:, None]
    return table[i0] * (1.0 - frac) + table[i0 + 1] * frac


def _build_program(reps=1, ndev=NCORES, stages="full", gps_reduce=False,
                   ident_first=False, lp_sums=False, dr=True,
                   io_bufs=4, ps_bufs=6):
    """stages: "dma" (input DMAs + const out DMA), "mm" (+matmuls,
    exp but no DVE), "full" (everything).  For HW bisection."""
    import concourse.mybir as mybir
    from concourse import bacc
    from concourse.tile import TileContext

    F32 = mybir.dt.float32
    F16 = mybir.dt.float16
    F8 = mybir.dt.float8e4
    U8 = mybir.dt.uint8
    nc = bacc.Bacc("TRN2", target_bir_lowering=False, debug=False,
                   num_devices=ndev)

    uv_d = nc.dram_tensor("uv", [128, BH * UV_BYTES], U8,
                          kind="ExternalInput")
    mm_d = nc.dram_tensor("mm", [128, 2 * L], F16, kind="ExternalInput")
    id_d = nc.dram_tensor("ident", [128, 128], F16, kind="ExternalInput")
    out_d = nc.dram_tensor("out", [128, BH * 2 * L], F16,
                           kind="ExternalOutput")
    tok_d = nc.dram_tensor("tok", [128, 2], F32, kind="ExternalOutput")

    with TileContext(nc) as tc:
        with (
            tc.tile_pool(name="const", bufs=1) as c_pool,
            tc.tile_pool(name="io", bufs=io_bufs) as io_pool,
            tc.tile_pool(name="ex", bufs=3) as ex_pool,
            tc.tile_pool(name="sm", bufs=3) as sm_pool,
            tc.tile_pool(name="outp", bufs=2) as out_pool,
            tc.tile_pool(name="psum", bufs=ps_bufs, space="PSUM") as ps_pool,
        ):
            mm_t = c_pool.tile([128, 2 * L], F16, tag="mm")
            nc.sync.dma_start(mm_t[:], mm_d[:])
            id_t = c_pool.tile([128, 128], F16, tag="id")
            nc.sync.dma_start(id_t[:], id_d[:])
            bias_t = c_pool.tile([128, 1], F32, tag="bias")
            nc.vector.memset(bias_t[:], EXP_BIAS)

            for _rep in range(reps):
                if stages == "full":
                    out_t = out_pool.tile([128, BH * 2 * L], F16, tag="out")
                uv_t = io_pool.tile([128, BH * UV_BYTES], U8, tag="uv")
                nc.sync.dma_start(uv_t[:], uv_d[:])
                if ident_first and stages == "full":
                    ps_list = []
                    for l in range(BH):
                        psl = ps_pool.tile([128, 2 * L], F32, tag="ps")
                        ps_list.append(psl)
                        nc.tensor.matmul(
                            psl[:], lhsT=id_t[:], rhs=mm_t[:],
                            start=True, stop=False, skip_group_check=True)
                for l in range(BH):
                    lo = l * UV_BYTES
                    if stages == "dma":
                        if l == BH - 1:
                            nc.sync.dma_start(
                                out_d[:].bitcast(U8)[:, :UV_BYTES],
                                uv_t[:, lo:lo + UV_BYTES])
                        continue
                    uth = uv_t[:, lo + UTH_OFF:lo + VTH_OFF].bitcast(F16)
                    vth = uv_t[:, lo + VTH_OFF:lo + UTQ_OFF].bitcast(F16)
                    utq = uv_t[:, lo + UTQ_OFF:lo + VTQ_OFF].bitcast(F8)
                    vtq = uv_t[:, lo + VTQ_OFF:lo + UV_BYTES].bitcast(F8)

                    ex_t = ex_pool.tile([128, 2 * L], F16, tag="ex")
                    sums = sm_pool.tile([128, 2], F16 if lp_sums else F32,
                                        tag="sums")
                    recip = sm_pool.tile([128, 2], F32, tag="recip")

                    if ident_first:
                        ps = ps_list[l]
                    else:
                        ps = ps_pool.tile([128, 2 * L], F32, tag="ps")
                        nc.tensor.matmul(
                            ps[:], lhsT=id_t[:], rhs=mm_t[:],
                            start=True, stop=False, skip_group_check=True)
                    for it in range(2):
                        for c in range(NF16):
                            nc.tensor.matmul(
                                ps[:, it * L:(it + 1) * L],
                                lhsT=uth[:, c * L + it * 128:
                                         c * L + it * 128 + 128],
                                rhs=vth[:, c * L:(c + 1) * L],
                                start=False,
                                stop=(not dr and False),
                                skip_group_check=True)
                        if dr:
                            # fp8 chunks 2+3 fused: K = 128 partitions x
                            # 2 k-tiles, one DoubleRow matmul
                            wv = utq.rearrange("p (kt i) -> p kt i", kt=2)
                            rv = vtq.rearrange("p (kt j) -> p kt j", kt=2)
                            nc.tensor.matmul(
                                ps[:, it * L:(it + 1) * L],
                                lhsT=wv[:, :, it * 128:it * 128 + 128],
                                rhs=rv[:, :, :],
                                start=False, stop=True,
                                perf_mode=mybir.MatmulPerfMode.DoubleRow,
                                skip_group_check=True)
                        else:
                            for c in range(NF16, NCH):
                                cc = c - NF16
                                nc.tensor.matmul(
                                    ps[:, it * L:(it + 1) * L],
                                    lhsT=utq[:, cc * L + it * 128:
                                             cc * L + it * 128 + 128],
                                    rhs=vtq[:, cc * L:(cc + 1) * L],
                                    start=False, stop=(c == NCH - 1),
                                    skip_group_check=True)
                    nc.scalar.activation(
                        ex_t[:], ps[:],
                        mybir.ActivationFunctionType.Exp,
                        bias=bias_t[:])
                    if stages == "mm":
                        if l == BH - 1:
                            nc.sync.dma_start(out_d[:, :2 * L], ex_t[:])
                        continue
                    red = nc.gpsimd if gps_reduce else nc.vector
                    if lp_sums:
                        with nc.allow_low_precision("fp16 softmax denom"):
                            red.reduce_sum(
                                sums[:],
                                ex_t[:].rearrange("p (s j) -> p s j", j=L),
                                axis=mybir.AxisListType.X)
                    else:
                        red.reduce_sum(
                            sums[:],
                            ex_t[:].rearrange("p (s j) -> p s j", j=L),
                            axis=mybir.AxisListType.X)
                    nc.vector.reciprocal(recip[:], sums[:])
                    for it in range(2):
                        nc.vector.tensor_scalar(
                            out_t[:, (l * 2 + it) * L:(l * 2 + it + 1) * L],
                            ex_t[:, it * L:(it + 1) * L],
                            recip[:, it:it + 1], OUT_SCALE,
                            op0=mybir.AluOpType.mult,
                            op1=mybir.AluOpType.mult)
                if stages == "full":
                    nc.sync.dma_start(out_d[:], out_t[:])
            # tiny completion token (one DMA after the last rep): lets the
            # bench wait on execution end by fetching ~1 KB instead of MBs
            if stages == "full":
                if lp_sums:
                    nc.sync.dma_start(tok_d[:].bitcast(F16)[:, :2], sums[:])
                else:
                    nc.sync.dma_start(tok_d[:], sums[:])
            elif stages == "mm":
                nc.sync.dma_start(tok_d[:].bitcast(F16), ex_t[:, :4])
            else:
                nc.sync.dma_start(tok_d[:].bitcast(U8), uv_t[:, :8])

    nc.compile()
    return nc


def _prep_core_inputs(q, k, mask, attention):
    """Host-side prep: packed factor tensors + premasked mask tile."""
    import concourse.mybir as mybir
    F8NP = mybir.dt.np(mybir.dt.float8e4)

    q = np.asarray(q, np.float32)
    k = np.asarray(k, np.float32)
    a = np.asarray(attention, np.float32).reshape(H, D)
    mask = np.asarray(mask).reshape(B, L, L)

    xm = float(max(np.abs(q).max(), np.abs(k).max())) + 0.05
    x, G, Hf = _factors(xm)

    gq = _interp_all(q.ravel(), x, G).reshape(B, H, L, D, R)
    hk = _interp_all(k.ravel(), x, Hf).reshape(B, H, L, D, R)

    in_maps = []
    for core in range(NCORES):
        uv = np.empty((128, BH * UV_BYTES), np.uint8)
        for l in range(BH):
            f = BH * core + l
            b, h = f // H, f % H
            # (R, D, L) -> rows (t*64+d), cols i
            UT = (gq[b, h] * a[h][None, :, None]).transpose(2, 1, 0)
            VT = hk[b, h].transpose(2, 1, 0)
            UT = UT.reshape(R * D, L)
            VT = VT.reshape(R * D, L)
            # fp16 chunks 0..NF16-1: (128, NF16*L) partition-major
            uth = np.concatenate(
                [UT[c * 128:(c + 1) * 128] for c in range(NF16)],
                axis=1).astype(np.float16)
            vth = np.concatenate(
                [VT[c * 128:(c + 1) * 128] for c in range(NF16)],
                axis=1).astype(np.float16)
            utq = np.concatenate(
                [UT[c * 128:(c + 1) * 128] for c in range(NF16, NCH)],
                axis=1).astype(F8NP)
            vtq = np.concatenate(
                [VT[c * 128:(c + 1) * 128] for c in range(NF16, NCH)],
                axis=1).astype(F8NP)
            uv[:, l * UV_BYTES + UTH_OFF:l * UV_BYTES + VTH_OFF] = uth.view(np.uint8)
            uv[:, l * UV_BYTES + VTH_OFF:l * UV_BYTES + UTQ_OFF] = vth.view(np.uint8)
            uv[:, l * UV_BYTES + UTQ_OFF:l * UV_BYTES + VTQ_OFF] = utq.view(np.uint8)
            uv[:, l * UV_BYTES + VTQ_OFF:l * UV_BYTES + UV_BYTES] = vtq.view(np.uint8)
        b0 = BH * core // H
        mb = np.where(mask[b0], np.float16(MASK_NEG), np.float16(0))
        mm = np.concatenate([mb[:128], mb[128:]], axis=1).astype(np.float16)
        in_maps.append({
            "uv": uv,
            "mm": np.ascontiguousarray(mm),
            "ident": np.eye(128, dtype=np.float16),
        })
    return in_maps


def _get_runner():
    """Persistent jitted shard_map runner over 8 cores."""
    if "runner" in _cache:
        return _cache["runner"]

    import jax
    import concourse.mybir as mybir
    from jax.sharding import Mesh, PartitionSpec
    from jax.experimental.shard_map import shard_map
    from concourse import bass2jax

    bass2jax.install_neuronx_cc_hook()
    nc = _build_program()

    part_name = (nc.partition_id_tensor.name
                 if nc.partition_id_tensor else None)
    in_names, out_names, out_avals, zero_outs = [], [], [], []
    for alloc in nc.m.functions[0].allocations:
        if not isinstance(alloc, mybir.MemoryLocationSet):
            continue
        name = alloc.memorylocations[0].name
        if alloc.kind == "ExternalInput":
            if name != part_name:
                in_names.append(name)
        elif alloc.kind == "ExternalOutput":
            shape = tuple(alloc.tensor_shape)
            dtype = mybir.dt.np(alloc.dtype)
            out_names.append(name)
            out_avals.append(jax.core.ShapedArray(shape, dtype))
            zero_outs.append(np.zeros(shape, dtype))
    n_params = len(in_names)
    all_names = in_names + out_names
    if part_name is not None:
        all_names = all_names + [part_name]

    def _body(*args):
        operands = list(args)
        if part_name is not None:
            operands.append(bass2jax.partition_id_tensor())
        return tuple(bass2jax._bass_exec_p.bind(
            *operands,
            out_avals=tuple(out_avals),
            in_names=tuple(all_names),
            out_names=tuple(out_names),
            lowering_input_output_aliases=(),
            sim_require_finite=True,
            sim_require_nnan=True,
            nc=nc,
        ))

    devices = jax.devices()[:NCORES]
    mesh = Mesh(np.asarray(devices), ("core",))
    n_outs = len(out_names)
    sharded = jax.jit(
        shard_map(_body, mesh=mesh,
                  in_specs=(PartitionSpec("core"),) * (n_params + n_outs),
                  out_specs=(PartitionSpec("core"),) * n_outs,
                  check_rep=False),
        donate_argnums=tuple(range(n_params, n_params + n_outs)),
        keep_unused=True)

    def run(in_maps):
        concat_in = [
            np.concatenate([in_maps[c][nm] for c in range(NCORES)], axis=0)
            for nm in in_names]
        concat_zeros = [np.zeros((NCORES * z.shape[0], *z.shape[1:]), z.dtype)
                        for z in zero_outs]
        outs = sharded(*concat_in, *concat_zeros)
        return [
            {nm: np.asarray(outs[i]).reshape(NCORES, *out_avals[i].shape)[c]
             for i, nm in enumerate(out_names)}
            for c in range(NCORES)]

    run.sharded = sharded
    run.in_names = in_names
    run.zero_outs = zero_outs
    _cache["runner"] = run
    return run


def kernel(q, k, scale, mask, attention):
    results = _get_runner()(_prep_core_inputs(q, k, mask, attention))
    attn = np.empty((B, H, L, L), np.float32)
    inv = np.float32(1.0 / OUT_SCALE)
    for core in range(NCORES):
        o = results[core]["out"]                  # (128, BH*2*L) f16
        of = o.astype(np.float32) * inv
        for l in range(BH):
            f = BH * core + l
            b, h = f // H, f % H
            attn[b, h, :128] = of[:, (l * 2) * L:(l * 2 + 1) * L]
            attn[b, h, 128:] = of[:, (l * 2 + 1) * L:(l * 2 + 2) * L]
    return attn
